# revision 2
# baseline (speedup 1.0000x reference)
"""Trainium2 Bass kernel v2.3 for linear attention (silu+1 feature map, cumsum
over T) with dense 1024x1024 in/out projections.

Sharding: 8 cores = 4 batches x 2 head-groups (8 heads / 512 channels each).
Each core: q/k/v projections for its 512 channels over T=4096, the linear-
attention recurrence, and a partial Wo (512 in-ch -> 1024 out-ch). Host sums
the two bf16 partials per batch, scales 1/64 and adds bo.

v2.3: pair-slabs of 1024 tokens (PSUM pair tiles + wide ACT reads), all
elementwise on DVE (DVE/GPSIMD share SBUF ports), phi_k*v folded into the
kvs scan's second operand, chunk-PAIRED DVE tiles so phq/pq/nm/at run as
[128, 2048] ops, den matmuls emitted right after pq with reciprocals
adjacent (no PSUM-bank stall), reciprocal head-broadcast via DRAM
round-trip DMA, Wo/out lagged two pair-slabs, out DMAs on the gpsimd
queue (drain spread over three queues).
"""

import numpy as np
import ml_dtypes

import concourse.bass as bass
import concourse.mybir as mybir
from concourse import bacc, tile
from concourse.bass_utils import run_bass_kernel_spmd

BF16 = mybir.dt.bfloat16
F32 = mybir.dt.float32
FP8 = mybir.dt.float8e4
DR = mybir.MatmulPerfMode.DoubleRow
XS = 0.125        # host scales x by XS, weights by 1/(XS*PS)
PS = 0.125        # ACT scale undoing the fp8 pre-scaling: psum*PS = true value
ADD = mybir.AluOpType.add
MULT = mybir.AluOpType.mult
BYPASS = mybir.AluOpType.bypass
SILU = mybir.ActivationFunctionType.Silu
COPY = mybir.ActivationFunctionType.Copy
IDENT = mybir.ActivationFunctionType.Identity

B, C, T = 4, 1024, 4096
H, DH = 16, 64
CG = 512            # channels per head-group (per core)
SP = 1024           # tokens per pair-slab
NPS = T // SP       # 4
NCH = CG // 128     # 4 chunks of 128 channels
NCC = NCH // 2      # 2 chunk-pairs
KCH = C // 128      # 8 input-channel chunks
MO = C // 128       # 8 output-channel chunks


def flat(ap):
    return ap.rearrange("p a b -> p (a b)")


def build():
    nc = bacc.Bacc(target_bir_lowering=False)

    x_d = nc.declare_dram_parameter("x", [C, T], FP8, isOutput=False)
    wq_d = nc.declare_dram_parameter("wq", [C, CG], FP8, isOutput=False)
    wk_d = nc.declare_dram_parameter("wk", [C, CG], FP8, isOutput=False)
    wv_d = nc.declare_dram_parameter("wv", [C, CG], FP8, isOutput=False)
    wo_d = nc.declare_dram_parameter("wo", [CG, C], FP8, isOutput=False)
    bq_d = nc.declare_dram_parameter("bq", [CG, 1], F32, isOutput=False)
    bv_d = nc.declare_dram_parameter("bv", [CG, 1], F32, isOutput=False)
    em_d = nc.declare_dram_parameter("emat", [CG, 8], BF16, isOutput=False)
    on_d = nc.declare_dram_parameter("ones", [128, SP], BF16, isOutput=False)
    scr_d = nc.declare_dram_parameter("scratch", [8 * NPS, SP], F32, isOutput=False)
    out_d = nc.declare_dram_parameter("out", [C, T], BF16, isOutput=True)

    with tile.TileContext(nc) as tc:
        from contextlib import ExitStack

        with ExitStack() as ctx:
            wpool = ctx.enter_context(tc.tile_pool(name="w", bufs=1))
            xpool = ctx.enter_context(tc.tile_pool(name="xp", bufs=2))
            ppool = ctx.enter_context(tc.tile_pool(name="proj", bufs=2, space="PSUM"))
            dpool = ctx.enter_context(tc.tile_pool(name="denp", bufs=1, space="PSUM"))
            wps = ctx.enter_context(tc.tile_pool(name="wops", bufs=3, space="PSUM"))
            a1pool = ctx.enter_context(tc.tile_pool(name="act1", bufs=1))
            a2pool = ctx.enter_context(tc.tile_pool(name="act2", bufs=2))
            spool = ctx.enter_context(tc.tile_pool(name="state", bufs=2))
            npool = ctx.enter_context(tc.tile_pool(name="nmp", bufs=3))
            rpool = ctx.enter_context(tc.tile_pool(name="rec", bufs=2))
            outpool = ctx.enter_context(tc.tile_pool(name="outp", bufs=5))

            wq_t = wk_t = wv_t = wo_t = None
            em_t = bq_t = bv_t = ones_t = None

            prev_ks = [None] * NCC   # chunk-pair tiles [128, 2, SP]
            prev_kvs = [None] * NCC
            prep = None    # (ps, at_l): at-muls done, Wo bundles due next iter
            fresh = None   # (ps, nm_l, rb_l): produced last iter

            def emit_at(ps, nm_l, rb_l):
                """at = nm * rb per chunk-pair (one [128,2048] fp8 op each)."""
                at_l = []
                for cc in range(NCC):
                    at = spool.tile([128, 2, SP], FP8, tag=f"at{cc}",
                                    name=f"at{ps}_{cc}")
                    nc.vector.tensor_mul(flat(at[:]), flat(nm_l[cc][:]),
                                         flat(rb_l[cc][:]))
                    at_l.append(at)
                return at_l

            def emit_wo_mm(ps, at_l, mo, h):
                hs = slice(512 * h, 512 * (h + 1))
                wo_ps = wps.tile([128, 512], F32, tag="wo", name=f"wo{ps}_{mo}_{h}")
                for kk in range(NCC):
                    nc.tensor.matmul(wo_ps[:], wo_t[kk][:, :, 128 * mo : 128 * (mo + 1)],
                                     at_l[kk][:, :, hs], start=(kk == 0),
                                     stop=(kk == NCC - 1), perf_mode=DR)
                return wo_ps

            def emit_wo_out(ps, wo_ps, mo, h, qeng=None, ceng=None):
                t0 = SP * ps
                ot = outpool.tile([128, 512], BF16, tag="ot", name=f"ot{ps}_{mo}_{h}")
                if ceng is nc.vector:
                    nc.vector.tensor_copy(ot[:], wo_ps[:])
                else:
                    nc.scalar.copy(ot[:], wo_ps[:])
                (qeng or nc.gpsimd).dma_start(
                    out_d[128 * mo : 128 * (mo + 1),
                          t0 + 512 * h : t0 + 512 * (h + 1)], ot[:])

            def head(s):
                nonlocal wq_t, wk_t, wv_t, wo_t, em_t, bq_t, bv_t, ones_t
                t0 = SP * s
                ts = slice(t0, t0 + SP)
                x_t = []
                for k in range(KCH // 2):
                    xt = xpool.tile([128, 2, SP], FP8, tag=f"x{k}", name=f"x{k}_{s}")
                    nc.sync.dma_start(
                        xt[:],
                        x_d[256 * k : 256 * (k + 1), ts].rearrange(
                            "(ko ki) t -> ki ko t", ko=2))
                    x_t.append(xt)
                if wq_t is None:
                    # startup: spread weight loads over engine queues
                    def load_w8(dram, tagp, n, fd, eng):
                        tiles = []
                        for k in range(n):
                            t = wpool.tile([128, 2, fd], FP8, tag=f"{tagp}{k}",
                                           name=f"{tagp}{k}")
                            eng.dma_start(
                                t[:],
                                dram[256 * k : 256 * (k + 1), :].rearrange(
                                    "(ko ki) m -> ki ko m", ko=2))
                            tiles.append(t)
                        return tiles

                    def load(pool, shape, dtype, src, tag, eng):
                        t = pool.tile(shape, dtype, tag=tag, name=tag)
                        eng.dma_start(t[:], src)
                        return t

                    wk_t = load_w8(wk_d, "wk", KCH // 2, CG, nc.scalar)
                    wv_t = load_w8(wv_d, "wv", KCH // 2, CG, nc.scalar)
                    wq_t = load_w8(wq_d, "wq", KCH // 2, CG, nc.gpsimd)
                    bq_t = [load(wpool, [128, 1], F32,
                                 bq_d[128 * c : 128 * (c + 1), :], f"bq{c}", nc.gpsimd)
                            for c in range(NCH)]
                    bv_t = [load(wpool, [128, 1], F32,
                                 bv_d[128 * c : 128 * (c + 1), :], f"bv{c}", nc.gpsimd)
                            for c in range(NCH)]
                    ones_t = load(wpool, [128, SP], BF16, on_d[:, :], "ones", nc.gpsimd)
                    em_t = [load(wpool, [128, 8], BF16,
                                 em_d[128 * c : 128 * (c + 1), :], f"em{c}", nc.gpsimd)
                            for c in range(NCH)]
                    wo_t = load_w8(wo_d, "wo", NCC, C, nc.gpsimd)

                # two-iterations-old pair-slab whose at tiles were already
                # computed at the end of the previous iteration
                at_l = None
                if prep is not None:
                    ps, at_l = prep

                pq_l = []   # chunk-pair tiles [128, 2, SP]
                nm_l = []
                late = []   # deferred kvs-scan/nm emission (not den-critical)
                rec32 = rpool.tile([8, SP], F32, tag="rec32", name=f"rec32_{s}")
                K2 = KCH // 2
                for cc in range(NCC):
                    sk2 = a2pool.tile([128, 2, SP], BF16, tag=f"sk{cc}", name=f"sk{s}_{cc}")
                    vs2 = a2pool.tile([128, 2, SP], BF16, tag=f"vs{cc}", name=f"vs{s}_{cc}")
                    sq2 = a1pool.tile([128, 2, SP], BF16, tag=f"sq{cc}", name=f"sq{s}_{cc}")
                    skv2 = a1pool.tile([128, 2, SP], BF16, tag=f"skv{cc}", name=f"skv{s}_{cc}")
                    ks2 = spool.tile([128, 2, SP], BF16, tag=f"ks{cc}", name=f"ks{s}_{cc}")
                    kvs2 = spool.tile([128, 2, SP], BF16, tag=f"kvs{cc}", name=f"kvs{s}_{cc}")
                    for j in range(2):
                        c = 2 * cc + j
                        cs = slice(128 * c, 128 * (c + 1))
                        # k, v, q projections into 2-bank pair tiles
                        ps_k = ppool.tile([128, 2, 512], F32, tag="proj", name=f"psk{s}_{c}")
                        for h in range(2):
                            hsl = slice(512 * h, 512 * (h + 1))
                            for k in range(K2):
                                nc.tensor.matmul(ps_k[:, h, :], wk_t[k][:, :, cs],
                                                 x_t[k][:, :, hsl], start=(k == 0),
                                                 stop=(k == K2 - 1), perf_mode=DR)
                        ps_v = ppool.tile([128, 2, 512], F32, tag="proj", name=f"psv{s}_{c}")
                        for h in range(2):
                            hsl = slice(512 * h, 512 * (h + 1))
                            for k in range(K2):
                                nc.tensor.matmul(ps_v[:, h, :], wv_t[k][:, :, cs],
                                                 x_t[k][:, :, hsl], start=(k == 0),
                                                 stop=(k == K2 - 1), perf_mode=DR)
                        ps_q = ppool.tile([128, 2, 512], F32, tag="proj", name=f"psq{s}_{c}")
                        for h in range(2):
                            hsl = slice(512 * h, 512 * (h + 1))
                            for k in range(K2):
                                nc.tensor.matmul(ps_q[:, h, :], wq_t[k][:, :, cs],
                                                 x_t[k][:, :, hsl], start=(k == 0),
                                                 stop=(k == K2 - 1), perf_mode=DR)

                        # one bundle of the (s-2) pair-slab's Wo matmuls
                        wo_bundle = []
                        if at_l is not None:
                            for mo in (2 * c, 2 * c + 1):
                                for h in range(2):
                                    wo_bundle.append(
                                        (ps, emit_wo_mm(ps, at_l, mo, h), mo, h))

                        # activations into pair-tile slices (wide PSUM reads)
                        nc.scalar.activation(sk2[:, j, :], flat(ps_k[:]),
                                             SILU, scale=PS)
                        nc.scalar.activation(vs2[:, j, :], flat(ps_v[:]),
                                             IDENT, bias=bv_t[c][:], scale=PS)
                        nc.scalar.activation(sq2[:, j, :], flat(ps_q[:]),
                                             SILU, bias=bq_t[c][:], scale=PS)

                        # drain the Wo bundle (ACT copy + DMA out)
                        for (ps2, wo_ps, mo, h) in wo_bundle:
                            emit_wo_out(ps2, wo_ps, mo, h)

                        # skv = silu_k * v (per chunk so the kvs scan isn't
                        # gated on the next chunk's activations)
                        nc.vector.tensor_mul(skv2[:, j, :], sk2[:, j, :], vs2[:, j, :])

                        # running cumsums: ks = cumsum(silu_k + 1),
                        # kvs = cumsum(skv + v); chained across pair-slabs
                        ik = 0.0 if s == 0 else prev_ks[cc][:, j, SP - 1 : SP]
                        nc.vector.tensor_tensor_scan(ks2[:, j, :], sk2[:, j, :],
                                                     ones_t[:], initial=ik,
                                                     op0=ADD, op1=ADD)
                        # kvs scan + nm feed the (s+2) tail only: defer them
                        # behind the den-critical ks->pq chain
                        ikv = 0.0 if s == 0 else prev_kvs[cc][:, j, SP - 1 : SP]
                        late.append((kvs2[:, j, :], skv2[:, j, :],
                                     vs2[:, j, :], ikv))
                    prev_ks[cc], prev_kvs[cc] = ks2, kvs2

                    # pair-wide: phi_q = silu_q + 1; pq = phi_q * ks
                    phq2 = a1pool.tile([128, 2, SP], BF16, tag=f"phq{cc}",
                                       name=f"phq{s}_{cc}")
                    nc.vector.tensor_scalar_add(flat(phq2[:]), flat(sq2[:]), 1.0)
                    pq2 = a1pool.tile([128, 2, SP], BF16, tag=f"pq{cc}",
                                      name=f"pq{s}_{cc}")
                    nc.vector.tensor_mul(flat(pq2[:]), flat(phq2[:]), flat(ks2[:]))
                    pq_l.append(pq2)

                    late.append((cc, phq2))

                # den (both halves) as soon as pq is ready; PE arrives here
                # at ~30us into its iteration, pq lands ~24us into DVE's
                for h in range(2):
                    den_ps = dpool.tile([8, 512], F32, tag="den", name=f"den{s}_{h}")
                    for c in range(NCH):
                        nc.tensor.matmul(den_ps[:], em_t[c][:],
                                         pq_l[c // 2][:, c % 2, 512 * h : 512 * (h + 1)],
                                         start=(c == 0), stop=(c == NCH - 1))
                    nc.vector.reciprocal_approx_fast(rec32[:, 512 * h : 512 * (h + 1)],
                                                     den_ps[:])
                nc.sync.dma_start(scr_d[8 * s : 8 * (s + 1), :], rec32[:])

                # deferred: kvs scans, then nm per chunk-pair
                kvs_by_cc = {}
                for item in late:
                    if len(item) == 4:
                        o, a, b, ikv = item
                        nc.vector.tensor_tensor_scan(o, a, b, initial=ikv,
                                                     op0=ADD, op1=ADD)
                for item in late:
                    if len(item) == 2:
                        cc, phq2 = item
                        nm2 = npool.tile([128, 2, SP], BF16, tag=f"nm{cc}",
                                         name=f"nm{s}_{cc}")
                        nc.vector.tensor_mul(flat(nm2[:]), flat(phq2[:]),
                                             flat(prev_kvs[cc][:]))
                        nm_l.append(nm2)

                # at-muls for the one-iteration-old slab, emitted here (huge
                # slack: consumed by PE bundles ~one full iteration later)
                if fresh is not None:
                    prep_new = (fresh[0], emit_at(fresh[0], fresh[1], fresh[2]))
                else:
                    prep_new = None

                # rb = per-head broadcast of 1/den via DRAM round trip
                # (same queue as the store -> ordered); chunk-pair tiles
                rb_l = []
                for cc in range(NCC):
                    rb = rpool.tile([128, 2, SP], F32, tag=f"rb{cc}", name=f"rb{s}_{cc}")
                    for j in range(2):
                        for k in range(2):
                            src = scr_d[8 * s + 4 * cc + 2 * j + k
                                        : 8 * s + 4 * cc + 2 * j + k + 1, :]
                            nc.sync.dma_start(rb[64 * k : 64 * (k + 1), j, :],
                                              src.broadcast_to([64, SP]))
                    rb_l.append(rb)
                return (s, nm_l, rb_l, prep_new)

            for s in range(NPS):
                res = head(s)
                prep = res[3]
                fresh = (res[0], res[1], res[2])

            # drain: two pending pair-slabs; at(s) on DVE overlaps PE wo(s-1);
            # out DMAs spread over three queues
            ps0, at0 = prep
            ps1 = fresh[0]
            at1 = emit_at(fresh[0], fresh[1], fresh[2])
            qengs = [nc.gpsimd, nc.sync, nc.scalar]
            cengs = [nc.scalar, nc.vector]
            qi = 0
            for (ps, at_l) in ((ps0, at0), (ps1, at1)):
                for mo in range(MO):
                    for h in range(2):
                        wo_ps = emit_wo_mm(ps, at_l, mo, h)
                        emit_wo_out(ps, wo_ps, mo, h, qeng=qengs[qi % 3],
                                    ceng=cengs[qi % 2])
                        qi += 1

    nc.compile()
    return nc


_NC_CACHE = {}


def _get_nc():
    if "nc" not in _NC_CACHE:
        _NC_CACHE["nc"] = build()
    return _NC_CACHE["nc"]


def make_in_maps(x, Wq, bq, Wk, Wv, bv, Wo, bo):
    bf = ml_dtypes.bfloat16
    f8 = ml_dtypes.float8_e4m3
    WS = 1.0 / (XS * PS)  # weight pre-scale so that psum * PS = W @ x exactly
    x3 = np.asarray(x, np.float32)[..., 0]                      # (B, C, T)
    E = np.zeros((CG, 8), np.float32)
    for ch in range(CG):
        E[ch, ch // DH] = 1.0
    ones = np.ones((128, SP), bf)
    in_maps = []
    for core in range(8):
        b, g = core // 2, core % 2
        sl = slice(CG * g, CG * (g + 1))
        in_maps.append({
            "x": np.clip(x3[b] * XS, -240, 240).astype(f8),
            "wq": np.clip(np.ascontiguousarray(np.asarray(Wq, np.float32)[sl, :].T) * WS, -240, 240).astype(f8),
            "wk": np.clip(np.ascontiguousarray(np.asarray(Wk, np.float32)[sl, :].T) * WS, -240, 240).astype(f8),
            "wv": np.clip(np.ascontiguousarray(np.asarray(Wv, np.float32)[sl, :].T) * WS, -240, 240).astype(f8),
            "wo": np.clip(np.ascontiguousarray((np.asarray(Wo, np.float32)[:, sl] * 0.125).T) * 64.0, -240, 240).astype(f8),
            "bq": np.asarray(bq, np.float32)[sl].reshape(CG, 1).copy(),
            "bv": np.asarray(bv, np.float32)[sl].reshape(CG, 1).copy(),
            "emat": E.astype(bf),
            "ones": ones,
            "scratch": np.zeros((8 * NPS, SP), np.float32),
        })
    return in_maps


def assemble(results, bo):
    out = np.empty((B, C, T, 1), np.float32)
    bo_f = np.asarray(bo, np.float32)[:, None]
    for b in range(B):
        p0 = np.asarray(results[2 * b]["out"], np.float32)
        p1 = np.asarray(results[2 * b + 1]["out"], np.float32)
        out[b, :, :, 0] = (p0 + p1) * (1.0 / 64.0) + bo_f
    return out


def kernel(x, Wq, bq, Wk, Wv, bv, Wo, bo):
    nc = _get_nc()
    in_maps = make_in_maps(x, Wq, bq, Wk, Wv, bv, Wo, bo)
    res = run_bass_kernel_spmd(nc, in_maps, core_ids=list(range(8)))
    return assemble(res.results, bo)
